# revision 5
# baseline (speedup 1.0000x reference)
"""Trainium2 Bass kernel for a 5x5 valid convolution over 96x96 images.

Reference computes x @ W.T where W is the [8464, 9216] conv-as-matmul
matrix (10 GFLOP dense).  We compute the convolution directly on the
tensor engine as 5 PSUM-accumulated banded matmuls (row-conv over the
image-row contraction, column shifts folded into the rhs access pattern):

    out[oi, b, oj] = sum_kj  B_kj.T @ X[:, b, oj+kj]
    B_kj[i, oi]    = K[i-oi, kj]   (banded Toeplitz)

The band matrix depends only on the 25-float kernel K, so it is
expanded on the host (like the reference's conv_mat) and passed as a
per-core bf16 input.  x is loaded fp32 in four DMAs (two per HWDGE
ring, shorter completion tails) and cast to bf16 on DVE; matmuls run
bf16 (PSUM accumulates fp32; rel-err budget 2e-2 vs ~2e-3 incurred).

Sharding: data-parallel over batch; each of the 8 cores convolves 8
images.  Raw Bass without a Block, hand-scheduled static DAG.  Stores
alternate rings per 2-image quarter and issue as soon as each PSUM
half is copied out.
"""

import sys

sys.path.insert(0, "/opt/trn_rl_repo")

import numpy as np
import ml_dtypes

import bass_rust
import concourse.bass as bass
import concourse.mybir as mybir
from concourse.bass_utils import run_bass_kernel_spmd

# Problem geometry (hardcoded per the task contract).
BATCH = 64
IN = 96           # input image side
KD = 5            # conv kernel side
OD = IN - KD + 1  # output side = 92
ISIZE = IN * IN   # 9216
OSIZE = OD * OD   # 8464
NCORES = 8
BPC = BATCH // NCORES  # images per core = 8
HALF = BPC // 2        # images per PSUM accumulation group = 4
QTR = BPC // 4         # images per store quarter = 2
PAIR = 2               # images per x-load DMA


def _ap(view, offset, dims):
    ap = view.copy()
    ap.offset = offset
    ap.ap = bass_rust.VecI64Pair(dims)
    return ap


def _build_program():
    nc = bass.Bass()
    dt = mybir.dt.float32
    bf = mybir.dt.bfloat16

    x_in = nc.declare_dram_parameter("x", [BPC, ISIZE], dt, isOutput=False)
    b_in = nc.declare_dram_parameter("b", [IN, KD * OD], bf, isOutput=False)
    y_out = nc.declare_dram_parameter("y", [BPC, OSIZE], dt, isOutput=True)

    from contextlib import ExitStack

    with ExitStack() as ctx:
        b_sb = ctx.enter_context(nc.sbuf_tensor("b_sb", [IN, KD, OD], bf))
        x_sb = ctx.enter_context(nc.sbuf_tensor("x_sb", [IN, BPC, IN], dt))
        x_bf = ctx.enter_context(nc.sbuf_tensor("x_bf", [IN, BPC, IN], bf))
        out_sb = ctx.enter_context(nc.sbuf_tensor("out_sb", [OD, BPC, OD], dt))
        ps0 = ctx.enter_context(nc.psum_tensor("ps0", [OD, HALF, OD], dt))
        ps1 = ctx.enter_context(nc.psum_tensor("ps1", [OD, HALF, OD], dt))
        sem = lambda n: ctx.enter_context(nc.semaphore(n))
        sem_b = sem("sem_b")            # band matrix -> b_sb
        sem_x = [sem(f"sem_x{i}") for i in range(4)]  # image pairs -> x_sb
        sem_xbf = sem("sem_xbf")        # x halves cast to bf16
        sem_mm = sem("sem_mm")          # psum group done
        sem_copy = sem("sem_copy")      # psum -> out_sb quarter done
        sem_y = sem("sem_y")            # out_sb -> y

        psums = [ps0, ps1]

        def x_load(engine, pair, s):
            engine.dma_start(
                out=x_sb[:, pair * PAIR : (pair + 1) * PAIR, :],
                in_=_ap(
                    x_in[:],
                    pair * PAIR * ISIZE,
                    [[IN, IN], [ISIZE, PAIR], [1, IN]],
                ),
            ).then_inc(s, 16)

        # ---- sync ring: band matrix (gates everything), x pairs 0 and 2
        nc.sync.dma_start(out=b_sb[:], in_=b_in[:]).then_inc(sem_b, 16)
        x_load(nc.sync, 0, sem_x[0])
        x_load(nc.sync, 2, sem_x[2])

        # ---- scalar ring: x pairs 1 and 3
        x_load(nc.scalar, 1, sem_x[1])
        x_load(nc.scalar, 3, sem_x[3])

        # ---- vector: bf16 casts per half, then psum quarter copies
        for h in range(2):
            nc.vector.wait_ge(sem_x[2 * h], 16)
            nc.vector.wait_ge(sem_x[2 * h + 1], 16)
            nc.vector.tensor_copy(
                x_bf[:, h * HALF : (h + 1) * HALF, :],
                x_sb[:, h * HALF : (h + 1) * HALF, :],
            ).then_inc(sem_xbf, 1)

        # ---- tensor: h-outer accumulated bf16 matmuls
        nc.tensor.wait_ge(sem_b, 16)
        for h in range(2):
            nc.tensor.wait_ge(sem_xbf, h + 1)
            for kj in range(KD):
                mm = nc.tensor.matmul(
                    psums[h][:],
                    b_sb[:, kj, :],
                    _ap(
                        x_bf[:],
                        h * HALF * IN + kj,
                        [[BPC * IN, IN], [IN, HALF], [1, OD]],
                    ),
                    start=(kj == 0),
                    stop=(kj == KD - 1),
                )
                if kj == KD - 1:
                    mm.then_inc(sem_mm, 1)

        # ---- vector: quarter copies psum -> out_sb (q covers images 2q..2q+1)
        for q in range(4):
            h, lo = q // 2, (q % 2) * QTR
            nc.vector.wait_ge(sem_mm, h + 1)
            nc.vector.tensor_copy(
                out_sb[:, q * QTR : (q + 1) * QTR, :],
                psums[h][:, lo : lo + QTR, :],
            ).then_inc(sem_copy, 1)

        # ---- stores: quarters alternate between the two HWDGE rings
        def store(engine, q):
            engine.wait_ge(sem_copy, q + 1)
            engine.dma_start(
                out=_ap(
                    y_out[:],
                    q * QTR * OSIZE,
                    [[OD, OD], [OSIZE, QTR], [1, OD]],
                ),
                in_=out_sb[:, q * QTR : (q + 1) * QTR, :],
            ).then_inc(sem_y, 16)

        store(nc.sync, 0)
        store(nc.scalar, 1)
        store(nc.sync, 2)
        store(nc.scalar, 3)
        # hold execution open until every store has landed
        nc.sync.wait_ge(sem_y, 64)

    return nc


def _band_matrix(k: np.ndarray) -> np.ndarray:
    """Pre-reversed banded Toeplitz: b[i, kj, oi] = K[i-oi, kj], as bf16."""
    b = np.zeros((IN, KD, OD), np.float32)
    oi = np.arange(OD)
    for t in range(KD):
        for kj in range(KD):
            b[oi + t, kj, oi] = k[t, kj]
    return b.reshape(IN, KD * OD).astype(ml_dtypes.bfloat16)


_NC = None


def kernel(x: np.ndarray, kernel: np.ndarray) -> np.ndarray:
    global _NC
    if _NC is None:
        _NC = _build_program()

    x = np.ascontiguousarray(x, dtype=np.float32)
    k = np.ascontiguousarray(kernel, dtype=np.float32)
    b = _band_matrix(k)
    in_maps = [
        {"x": x[c * BPC : (c + 1) * BPC], "b": b} for c in range(NCORES)
    ]
    res = run_bass_kernel_spmd(_NC, in_maps, list(range(NCORES)))
    return np.concatenate([res.results[c]["y"] for c in range(NCORES)], axis=0)
